# revision 24
# baseline (speedup 1.0000x reference)
"""Trainium2 Bass kernel for causal self-attention (nn_CausalSelfAttention).

Problem (hardcoded):
    x:     [1, 4096, 1024] f32
    w_qkv: [1024, 3072] f32, b_qkv: [3072] f32
    w_out: [1024, 1024] f32, b_out: [1024] f32
    16 heads, head_dim 64, causal softmax attention.

Sharding: tensor-parallel over heads. 8 cores x 2 heads each. Each core
computes QKV for its heads, T^2 causal attention, and a partial output
projection in bf16; host sums the 8 partial projections and adds biases.

Math notes (exact simplifications, same as the f32r baseline):
  - b_k drops out (softmax shift invariance); b_v reduces to a host-side
    constant row b_v @ w_out; b_q applied on-device per partition.
  - Per-token softmax denominators obtained via a ones-column in V'.

v3 design (measured-HW-driven):
  - All matmul operands bf16 (1 cycle/row at 2.4 GHz sustained; LDWEIGHTS
    fully hidden). PSUM accumulation stays f32.
  - TQ=256 query tiles (16 tiles). Scores computed transposed S^T[keys, q]
    into [128, 2 chunks, 2 heads, 256] PSUM tiles (2 banks), exp'd by ONE
    ACT instruction per key-chunk pair (1024 cols) -> ex bf16 in SBUF.
  - AV uses ex chunks as *stationaries* (O[q,d] orientation): per
    (kchunk, qsub, head) one matmul of N=65 (V' moving with ones column)
    accumulating O + denominator in a single [128, 2, 2, 65] PSUM tile.
    This streams half the rows of the S^T-moving orientation.
  - Normalization: per-partition reciprocal+tensor_scalar on DVE (free vs
    the old sel-matmul broadcast), then PE-transpose O_norm -> O^T for the
    output projection (otn stationary, w_out moving, N=512 halves).
  - QKV of tile j+1 and norm/outproj of tile j-1 are interleaved into the
    attention pair loop of tile j so PE (~147us) and ACT exp (~146us)
    overlap instead of alternating.
  - y written bf16 (DVE cast from PSUM); host sums partials in f32.
"""

import numpy as np
import ml_dtypes

T = 4096
E = 1024
NCORES = 8
D = 64  # head dim
TQ = 256  # query tile
NJ = T // TQ  # 16
NKC = T // 128  # 32 key chunks

_CACHE = {}

# Results of the last SPMD run (exec_time_ns etc.), for the local test harness.
LAST_RESULTS = None


def _build():
    import concourse.bacc as bacc
    import concourse.tile as tile
    import concourse.mybir as mybir

    f32 = mybir.dt.float32
    bf16 = mybir.dt.bfloat16
    EXP = mybir.ActivationFunctionType.Exp

    nc = bacc.Bacc("TRN2", target_bir_lowering=False, debug=False)

    xT = nc.dram_tensor("xT", [E, T], bf16, kind="ExternalInput").ap()
    # per-core slice of w_qkv: cols [q(128) | k(128) | v(128)] for this core's
    # two heads
    wqkv = nc.dram_tensor("wqkv", [E, 384], bf16, kind="ExternalInput").ap()
    bq2 = nc.dram_tensor("bq2", [64, 2], f32, kind="ExternalInput").ap()
    wo = nc.dram_tensor("wo", [128, E], bf16, kind="ExternalInput").ap()
    mask_dram = nc.dram_tensor("mask", [128, 128], bf16, kind="ExternalInput").ap()
    ident_dram = nc.dram_tensor("ident", [128, 128], bf16, kind="ExternalInput").ap()
    y = nc.dram_tensor("y", [T, E], bf16, kind="ExternalOutput").ap()

    import os as _os
    with tile.TileContext(nc) as tc:
        with (
            tc.tile_pool(name="consts", bufs=1) as consts,
            tc.tile_pool(name="w", bufs=8) as wpool,
            tc.tile_pool(name="xt", bufs=2) as xtp,
            tc.tile_pool(name="qt", bufs=2) as qtp,
            tc.tile_pool(name="kt", bufs=NJ) as ktp,
            tc.tile_pool(name="v", bufs=NKC) as vp,
            tc.tile_pool(name="ex", bufs=6) as exp_p,
            tc.tile_pool(name="on", bufs=2) as onp,
            tc.tile_pool(name="otn", bufs=2) as otnp,
            tc.tile_pool(name="rd", bufs=8) as rdp,
            tc.tile_pool(name="ysb", bufs=3) as ysp,
            tc.tile_pool(name="st_ps", bufs=2, space="PSUM") as stp,
            tc.tile_pool(name="av_ps", bufs=2, space="PSUM") as avp,
            tc.tile_pool(name="mm_ps", bufs=2, space="PSUM") as mmp,
        ):
            # ---- constants (xT tile 0 is DMA'd first, see below; wo_sb
            # is only needed from tile 1 so it loads last) ----
            _xT3 = xT.rearrange("(e p) t -> p e t", e=8)
            _rr = [nc.sync, nc.gpsimd, nc.scalar]
            xt0 = xtp.tile([128, 8, TQ], bf16, name="xt0")
            w_sb = []
            for e in range(8):
                w = wpool.tile([128, 384], bf16)
                _rr[e % 3].dma_start(w[:], wqkv[128 * e : 128 * (e + 1), :])
                w_sb.append(w)
            for e in range(8):
                _rr[e % 3].dma_start(xt0[:, e, :], _xT3[:, e, 0:TQ])
            mask = consts.tile([128, 128], bf16)  # 1 where q >= k else 0
            nc.sync.dma_start(mask[:], mask_dram[:])
            ident = consts.tile([128, 128], bf16)
            nc.sync.dma_start(ident[:], ident_dram[:])
            bq_sb = consts.tile([64, 2], f32)
            nc.sync.dma_start(bq_sb[:], bq2[:])
            wo_sb = consts.tile([128, E], bf16)

            kt_tiles = [None] * NJ
            v_tiles = [None] * NKC
            qt_cur = [None]  # qt of the tile currently in attention
            xts_cur = [None]  # xt tiles for the next QKV phase

            xT3 = xT.rearrange("(e p) t -> p e t", e=8)

            def emit_xt_dma(j):
                if j == 0:
                    xts_cur[0] = xt0
                    return
                t0 = TQ * j
                xt = xtp.tile([128, 8, TQ], bf16)
                nc.sync.dma_start(xt[:], xT3[:, :, t0 : t0 + TQ])
                xts_cur[0] = xt

            def qkv_units(j):
                """Closures emitting QKV for tile j: [Q, K, V0, V1].

                Q^T/K^T are built per head at partition base 0: bf16 matmuls
                with base-64 operands + column-offset PSUM outputs fault on
                HW, so everything reads partitions 0-63 instead."""
                xt3 = xts_cur[0]

                def unit_qk(col0, out_tiles, bias):
                    # combined M=128 matmul (both heads), then partition-
                    # shifted DVE copies into per-head base-0 tiles
                    def emit():
                        ps = mmp.tile([128, TQ], f32, tag="mm")
                        for e in range(8):
                            nc.tensor.matmul(
                                ps[:], w_sb[e][:, col0 : col0 + 128], xt3[:, e, :],
                                start=(e == 0), stop=(e == 7),
                            )
                        for h in range(2):
                            t = out_tiles[h]
                            if bias is None:
                                nc.vector.tensor_copy(
                                    t[:], ps[64 * h : 64 * h + 64, :]
                                )
                            else:
                                nc.vector.tensor_scalar_add(
                                    t[:], ps[64 * h : 64 * h + 64, :],
                                    bias[:, h : h + 1],
                                )
                    return emit

                def unit_q():
                    qts = [qtp.tile([64, TQ], bf16, tag=f"qt_h{h}", name=f"qt{j}_{h}") for h in range(2)]
                    unit_qk(0, qts, bq_sb)()
                    qt_cur[0] = qts

                def unit_k():
                    kts = [ktp.tile([64, TQ], bf16, tag=f"kt_h{h}", name=f"kt{j}_{h}") for h in range(2)]
                    unit_qk(128, kts, None)()
                    kt_tiles[j] = kts

                def unit_v(c):
                    def emit():
                        ps = mmp.tile([128, 128], f32, tag="mm")
                        for e in range(8):
                            nc.tensor.matmul(
                                ps[:], xt3[:, e, 128 * c : 128 * (c + 1)],
                                w_sb[e][:, 256:384],
                                start=(e == 0), stop=(e == 7),
                            )
                        vt = vp.tile([128, 130], bf16)
                        nc.vector.memset(vt[:, 64:65], 1.0)
                        nc.vector.memset(vt[:, 129:130], 1.0)
                        nc.vector.tensor_copy(vt[:, 0:64], ps[:, 0:64])
                        nc.vector.tensor_copy(vt[:, 65:129], ps[:, 64:128])
                        v_tiles[2 * j + c] = vt
                    return emit

                return [unit_q, unit_k, unit_v(0), unit_v(1)]

            def finish_units(j, ontile, avtile):
                """Closures for tile j's normalize + transpose + outproj.

                ontile: SBUF tile for O_norm, avtile: the AV PSUM tile."""
                t0 = TQ * j
                av4 = avtile.rearrange("p (s h n) -> p s h n", s=2, h=2)
                on4 = ontile.rearrange("p (s h n) -> p s h n", s=2, h=2)

                def unit_norm():
                    # per-partition normalization: O_norm = O * (1/denom)
                    for s in range(2):
                        for h in range(2):
                            bb = rdp.tile([128, 1], f32, tag="rd")
                            nc.vector.reciprocal_approx_fast(
                                bb[:], av4[:, s, h, 64:65]
                            )
                            nc.vector.tensor_scalar_mul(
                                on4[:, s, h, 0:64], av4[:, s, h, 0:64], bb[:, 0:1]
                            )

                def unit_proj(s):
                    def emit():
                        # transpose O_norm[s] -> otn columns, both heads
                        otn = otnp.tile([128, 128], bf16, tag="otn")
                        for h in range(2):
                            pst = mmp.tile([64, 128], bf16, tag="mm")
                            nc.tensor.transpose(
                                pst[:], on4[:, s, h, 0:64], ident[:]
                            )
                            nc.vector.tensor_copy(
                                otn[64 * h : 64 * h + 64, :], pst[:]
                            )
                        ys = ysp.tile([128, E], bf16, tag="ys")
                        r0 = t0 + 128 * s
                        for half in range(2):
                            yp = mmp.tile([128, 512], f32, tag="mm")
                            nc.tensor.matmul(
                                yp[:], otn[:],
                                wo_sb[:, 512 * half : 512 * (half + 1)],
                                start=True, stop=True,
                            )
                            for q in range(2):
                                nc.vector.tensor_copy(
                                    ys[:, 512 * half + 256 * q : 512 * half + 256 * (q + 1)],
                                    yp[:, 256 * q : 256 * (q + 1)],
                                )
                            nc.sync.dma_start(
                                y[r0 : r0 + 128, 512 * half : 512 * (half + 1)],
                                ys[:, 512 * half : 512 * (half + 1)],
                            )
                    return emit

                return [unit_norm, unit_proj(0), unit_proj(1)]

            # ---- main loop ----
            import os as _os
            _njl = int(_os.environ.get("NJ_LIMIT", NJ))

            emit_xt_dma(0)
            nc.sync.dma_start(wo_sb[:], wo[:])
            for u in qkv_units(0):
                u()
            pending = []  # closures to interleave into the attention pairs

            for j in range(_njl):
                if j + 1 < _njl:
                    emit_xt_dma(j + 1)
                    pending = qkv_units(j + 1) + pending
                qt = qt_cur[0]

                # full-bank tile: PSUM start_tensor_calc zeroes whole 2KB banks,
                # so the AV accumulator gets a bank to itself. 4 groups
                # (s, h) share it; only the very first matmul uses start=True.
                av = avp.tile([128, 512], f32)
                av4 = av.rearrange("p (s h n) -> p s h n", s=2, h=2)  # n=128
                prev_ex = []  # [(ex4, pair index)] awaiting AV emission (depth 2)

                def emit_av(ex4, p):
                    for pc in range(2):
                        c = 2 * p + pc
                        for s in range(2):
                            if p == j and pc == 1 and s == 0:
                                continue  # block above the diagonal
                            for h in range(2):
                                nc.tensor.matmul(
                                    av4[:, s, h, 0:65],
                                    ex4[:, pc, h, 128 * s : 128 * s + 128],
                                    v_tiles[c][:, 65 * h : 65 * h + 65],
                                    start=(c == 0 and s == 0 and h == 0),
                                    stop=(c == 2 * j + 1 - (1 - s)),
                                    skip_group_check=True,
                                )

                for p in range(j + 1):
                    # scores for key chunks 2p, 2p+1 (both heads)
                    st = stp.tile([128, 2 * 2 * TQ], f32)
                    st4 = st.rearrange("p (c h n) -> p c h n", c=2, h=2)
                    for pc in range(2):
                        c = 2 * p + pc
                        n0 = 128 if (p == j and pc == 1) else 0
                        jj, cc = divmod(c, 2)
                        for h in range(2):
                            # h0 starts (bank-wide zero); h1 lands on
                            # pending-zero bytes of the same bank
                            nc.tensor.matmul(
                                st4[:, pc, h, n0:TQ],
                                kt_tiles[jj][h][:, 128 * cc : 128 * cc + 128],
                                qt[h][:, n0:TQ],
                                start=(h == 0), stop=True,
                                skip_group_check=True,
                            )
                    ex = exp_p.tile([128, 2 * 2 * TQ], bf16)
                    ex4 = ex.rearrange("p (c h n) -> p c h n", c=2, h=2)
                    if p == j:
                        nc.scalar.activation(
                            ex4[:, 0, :, :], st4[:, 0, :, :], EXP, scale=0.125
                        )
                        nc.scalar.activation(
                            ex4[:, 1, :, 128:TQ], st4[:, 1, :, 128:TQ],
                            EXP, scale=0.125,
                        )
                        # mask the two diagonal blocks per head
                        for h in range(2):
                            nc.vector.tensor_mul(
                                ex4[:, 0, h, 0:128], ex4[:, 0, h, 0:128], mask[:]
                            )
                            nc.vector.tensor_mul(
                                ex4[:, 1, h, 128:TQ], ex4[:, 1, h, 128:TQ], mask[:]
                            )
                    else:
                        nc.scalar.activation(ex[:], st[:], EXP, scale=0.125)

                    prev_ex.append((ex4, p))
                    if len(prev_ex) > 2:
                        emit_av(*prev_ex.pop(0))
                    # spread interleaved units out; force-drain near tile end
                    if pending and (p % 2 == 0 or len(pending) >= (j - p)):
                        pending.pop(0)()

                while prev_ex:
                    emit_av(*prev_ex.pop(0))
                while pending:
                    pending.pop(0)()

                on = onp.tile([128, 2 * 2 * 64], bf16)
                pending = finish_units(j, on, av)

            while pending:
                pending.pop(0)()

    nc.compile()
    return nc


def _prep_inputs(x, w_qkv, b_qkv, w_out, b_out):
    x = np.asarray(x, dtype=np.float32).reshape(T, E)
    w_qkv = np.asarray(w_qkv, dtype=np.float32)
    b_qkv = np.asarray(b_qkv, dtype=np.float32)
    w_out = np.asarray(w_out, dtype=np.float32)
    b_out = np.asarray(b_out, dtype=np.float32)

    xT = np.ascontiguousarray(x.T).astype(ml_dtypes.bfloat16)
    mask = np.triu(np.ones((128, 128), dtype=np.float32)).astype(ml_dtypes.bfloat16)
    ident = np.eye(128, dtype=np.float32).astype(ml_dtypes.bfloat16)

    in_maps = []
    for cidx in range(NCORES):
        lo, hi = 128 * cidx, 128 * (cidx + 1)
        wq = w_qkv[:, lo:hi]
        wk = w_qkv[:, E + lo : E + hi]
        wv = w_qkv[:, 2 * E + lo : 2 * E + hi]
        wqkv_c = np.ascontiguousarray(
            np.concatenate([wq, wk, wv], axis=1)
        ).astype(ml_dtypes.bfloat16)
        in_maps.append(
            {
                "xT": xT,
                "wqkv": wqkv_c,
                "bq2": np.ascontiguousarray(b_qkv[lo:hi].reshape(2, 64).T),
                "wo": np.ascontiguousarray(w_out[lo:hi, :]).astype(
                    ml_dtypes.bfloat16
                ),
                "mask": mask,
                "ident": ident,
            }
        )
    # host-side constant: b_out plus the exact b_v contribution
    b_v = b_qkv[2 * E : 3 * E]
    const_row = b_out + b_v @ w_out
    return in_maps, const_row


def kernel(x, w_qkv, b_qkv, w_out, b_out):
    global LAST_RESULTS
    from concourse.bass_utils import run_bass_kernel_spmd

    if "nc" not in _CACHE:
        _CACHE["nc"] = _build()
    nc = _CACHE["nc"]

    in_maps, const_row = _prep_inputs(x, w_qkv, b_qkv, w_out, b_out)
    res = run_bass_kernel_spmd(nc, in_maps, core_ids=list(range(NCORES)))
    LAST_RESULTS = res

    out = np.zeros((T, E), dtype=np.float32)
    for r in res.results:
        out += r["y"].astype(np.float32)
    out += const_row[None, :].astype(np.float32)
    return out.reshape(1, T, E)


# revision 25
# speedup vs baseline: 1.0182x; 1.0182x over previous
"""Trainium2 Bass kernel for causal self-attention (nn_CausalSelfAttention).

Problem (hardcoded):
    x:     [1, 4096, 1024] f32
    w_qkv: [1024, 3072] f32, b_qkv: [3072] f32
    w_out: [1024, 1024] f32, b_out: [1024] f32
    16 heads, head_dim 64, causal softmax attention.

Sharding: tensor-parallel over heads. 8 cores x 2 heads each. Each core
computes QKV for its heads, T^2 causal attention, and a partial output
projection in bf16; host sums the 8 partial projections and adds biases.

Math notes (exact simplifications, same as the f32r baseline):
  - b_k drops out (softmax shift invariance); b_v reduces to a host-side
    constant row b_v @ w_out; b_q applied on-device per partition.
  - Per-token softmax denominators obtained via a ones-column in V'.

v3 design (measured-HW-driven):
  - All matmul operands bf16 (1 cycle/row at 2.4 GHz sustained; LDWEIGHTS
    fully hidden). PSUM accumulation stays f32.
  - TQ=256 query tiles (16 tiles). Scores computed transposed S^T[keys, q]
    into [128, 2 chunks, 2 heads, 256] PSUM tiles (2 banks), exp'd by ONE
    ACT instruction per key-chunk pair (1024 cols) -> ex bf16 in SBUF.
  - AV uses ex chunks as *stationaries* (O[q,d] orientation): per
    (kchunk, qsub, head) one matmul of N=65 (V' moving with ones column)
    accumulating O + denominator in a single [128, 2, 2, 65] PSUM tile.
    This streams half the rows of the S^T-moving orientation.
  - Normalization: per-partition reciprocal+tensor_scalar on DVE (free vs
    the old sel-matmul broadcast), then PE-transpose O_norm -> O^T for the
    output projection (otn stationary, w_out moving, N=512 halves).
  - QKV of tile j+1 and norm/outproj of tile j-1 are interleaved into the
    attention pair loop of tile j so PE (~147us) and ACT exp (~146us)
    overlap instead of alternating.
  - y written bf16 (DVE cast from PSUM); host sums partials in f32.
"""

import numpy as np
import ml_dtypes

T = 4096
E = 1024
NCORES = 8
D = 64  # head dim
TQ = 256  # query tile
NJ = T // TQ  # 16
NKC = T // 128  # 32 key chunks

_CACHE = {}

# Results of the last SPMD run (exec_time_ns etc.), for the local test harness.
LAST_RESULTS = None


def _build():
    import concourse.bacc as bacc
    import concourse.tile as tile
    import concourse.mybir as mybir

    f32 = mybir.dt.float32
    bf16 = mybir.dt.bfloat16
    EXP = mybir.ActivationFunctionType.Exp

    nc = bacc.Bacc("TRN2", target_bir_lowering=False, debug=False)

    xT = nc.dram_tensor("xT", [E, T], bf16, kind="ExternalInput").ap()
    # per-core slice of w_qkv: cols [q(128) | k(128) | v(128)] for this core's
    # two heads
    wqkv = nc.dram_tensor("wqkv", [E, 384], bf16, kind="ExternalInput").ap()
    bq2 = nc.dram_tensor("bq2", [64, 2], f32, kind="ExternalInput").ap()
    wo = nc.dram_tensor("wo", [128, E], bf16, kind="ExternalInput").ap()
    mask_dram = nc.dram_tensor("mask", [128, 128], bf16, kind="ExternalInput").ap()
    ident_dram = nc.dram_tensor("ident", [128, 128], bf16, kind="ExternalInput").ap()
    y = nc.dram_tensor("y", [T, E], bf16, kind="ExternalOutput").ap()

    import os as _os
    with tile.TileContext(nc) as tc:
        with (
            tc.tile_pool(name="consts", bufs=1) as consts,
            tc.tile_pool(name="w", bufs=8) as wpool,
            tc.tile_pool(name="xt", bufs=2) as xtp,
            tc.tile_pool(name="qt", bufs=2) as qtp,
            tc.tile_pool(name="kt", bufs=NJ) as ktp,
            tc.tile_pool(name="v", bufs=NKC) as vp,
            tc.tile_pool(name="ex", bufs=6) as exp_p,
            tc.tile_pool(name="on", bufs=2) as onp,
            tc.tile_pool(name="otn", bufs=2) as otnp,
            tc.tile_pool(name="rd", bufs=8) as rdp,
            tc.tile_pool(name="ysb", bufs=3) as ysp,
            tc.tile_pool(name="st_ps", bufs=2, space="PSUM") as stp,
            tc.tile_pool(name="av_ps", bufs=2, space="PSUM") as avp,
            tc.tile_pool(name="mm_ps", bufs=2, space="PSUM") as mmp,
        ):
            # ---- constants (xT tile 0 is DMA'd first, see below; wo_sb
            # is only needed from tile 1 so it loads last) ----
            _xT3 = xT.rearrange("(e p) t -> p e t", e=8)
            _rr = [nc.sync, nc.gpsimd, nc.scalar]
            xt0 = xtp.tile([128, 8, TQ], bf16, name="xt0")
            w_sb = []
            for e in range(8):
                w = wpool.tile([128, 384], bf16)
                _rr[e % 3].dma_start(w[:], wqkv[128 * e : 128 * (e + 1), :])
                w_sb.append(w)
            for e in range(8):
                _rr[e % 3].dma_start(xt0[:, e, :], _xT3[:, e, 0:TQ])
            mask = consts.tile([128, 128], bf16)  # 1 where q >= k else 0
            nc.sync.dma_start(mask[:], mask_dram[:])
            ident = consts.tile([128, 128], bf16)
            nc.sync.dma_start(ident[:], ident_dram[:])
            bq_sb = consts.tile([64, 2], f32)
            nc.sync.dma_start(bq_sb[:], bq2[:])
            wo_sb = consts.tile([128, E], bf16)

            kt_tiles = [None] * NJ
            v_tiles = [None] * NKC
            qt_cur = [None]  # qt of the tile currently in attention
            xts_cur = [None]  # xt tiles for the next QKV phase

            xT3 = xT.rearrange("(e p) t -> p e t", e=8)

            def emit_xt_dma(j):
                if j == 0:
                    xts_cur[0] = xt0
                    return
                t0 = TQ * j
                xt = xtp.tile([128, 8, TQ], bf16)
                nc.sync.dma_start(xt[:], xT3[:, :, t0 : t0 + TQ])
                xts_cur[0] = xt

            def qkv_units(j):
                """Closures emitting QKV for tile j: [Q, K, V0, V1].

                Q^T/K^T are built per head at partition base 0: bf16 matmuls
                with base-64 operands + column-offset PSUM outputs fault on
                HW, so everything reads partitions 0-63 instead."""
                xt3 = xts_cur[0]

                def unit_qk(col0, out_tiles, bias):
                    # combined M=128 matmul (both heads), then partition-
                    # shifted DVE copies into per-head base-0 tiles
                    def emit():
                        ps = mmp.tile([128, TQ], f32, tag="mm")
                        for e in range(8):
                            nc.tensor.matmul(
                                ps[:], w_sb[e][:, col0 : col0 + 128], xt3[:, e, :],
                                start=(e == 0), stop=(e == 7),
                            )
                        for h in range(2):
                            t = out_tiles[h]
                            if bias is None:
                                nc.vector.tensor_copy(
                                    t[:], ps[64 * h : 64 * h + 64, :]
                                )
                            else:
                                nc.vector.tensor_scalar_add(
                                    t[:], ps[64 * h : 64 * h + 64, :],
                                    bias[:, h : h + 1],
                                )
                    return emit

                def unit_q():
                    qts = [qtp.tile([64, TQ], bf16, tag=f"qt_h{h}", name=f"qt{j}_{h}") for h in range(2)]
                    unit_qk(0, qts, bq_sb)()
                    qt_cur[0] = qts

                def unit_k():
                    kts = [ktp.tile([64, TQ], bf16, tag=f"kt_h{h}", name=f"kt{j}_{h}") for h in range(2)]
                    unit_qk(128, kts, None)()
                    kt_tiles[j] = kts

                def unit_v(c):
                    def emit():
                        ps = mmp.tile([128, 128], f32, tag="mm")
                        for e in range(8):
                            nc.tensor.matmul(
                                ps[:], xt3[:, e, 128 * c : 128 * (c + 1)],
                                w_sb[e][:, 256:384],
                                start=(e == 0), stop=(e == 7),
                            )
                        vt = vp.tile([128, 130], bf16)
                        nc.vector.memset(vt[:, 64:65], 1.0)
                        nc.vector.memset(vt[:, 129:130], 1.0)
                        nc.vector.tensor_copy(vt[:, 0:64], ps[:, 0:64])
                        nc.vector.tensor_copy(vt[:, 65:129], ps[:, 64:128])
                        v_tiles[2 * j + c] = vt
                    return emit

                return [unit_q, unit_k, unit_v(0), unit_v(1)]

            def finish_units(j, ontile, avtile):
                """Closures for tile j's normalize + transpose + outproj.

                ontile: SBUF tile for O_norm, avtile: the AV PSUM tile."""
                t0 = TQ * j
                av4 = avtile.rearrange("p (s h n) -> p s h n", s=2, h=2)
                on4 = ontile.rearrange("p (s h n) -> p s h n", s=2, h=2)

                def unit_norm():
                    # per-partition normalization: O_norm = O * (1/denom)
                    for s in range(2):
                        for h in range(2):
                            bb = rdp.tile([128, 1], f32, tag="rd")
                            nc.vector.reciprocal_approx_fast(
                                bb[:], av4[:, s, h, 64:65]
                            )
                            nc.vector.tensor_scalar_mul(
                                on4[:, s, h, 0:64], av4[:, s, h, 0:64], bb[:, 0:1]
                            )

                def unit_proj(s):
                    def emit():
                        # transpose O_norm[s] -> otn columns, both heads
                        otn = otnp.tile([128, 128], bf16, tag="otn")
                        for h in range(2):
                            pst = mmp.tile([64, 128], bf16, tag="mm")
                            nc.tensor.transpose(
                                pst[:], on4[:, s, h, 0:64], ident[:]
                            )
                            nc.vector.tensor_copy(
                                otn[64 * h : 64 * h + 64, :], pst[:]
                            )
                        ys = ysp.tile([128, E], bf16, tag="ys")
                        for half in range(2):
                            yp = mmp.tile([128, 512], f32, tag="mm")
                            nc.tensor.matmul(
                                yp[:], otn[:],
                                wo_sb[:, 512 * half : 512 * (half + 1)],
                                start=True, stop=True,
                            )
                            nc.vector.tensor_copy(
                                ys[:, 512 * half : 512 * (half + 1)], yp[:]
                            )
                        r0 = t0 + 128 * s
                        nc.sync.dma_start(y[r0 : r0 + 128, :], ys[:])
                    return emit

                return [unit_norm, unit_proj(0), unit_proj(1)]

            # ---- main loop ----
            import os as _os
            _njl = int(_os.environ.get("NJ_LIMIT", NJ))

            emit_xt_dma(0)
            nc.sync.dma_start(wo_sb[:], wo[:])
            for u in qkv_units(0):
                u()
            pending = []  # closures to interleave into the attention pairs

            for j in range(_njl):
                if j + 1 < _njl:
                    emit_xt_dma(j + 1)
                    pending = qkv_units(j + 1) + pending
                qt = qt_cur[0]

                # full-bank tile: PSUM start_tensor_calc zeroes whole 2KB banks,
                # so the AV accumulator gets a bank to itself. 4 groups
                # (s, h) share it; only the very first matmul uses start=True.
                av = avp.tile([128, 512], f32)
                av4 = av.rearrange("p (s h n) -> p s h n", s=2, h=2)  # n=128
                prev_ex = []  # [(ex4, pair index)] awaiting AV emission (depth 2)

                def emit_av(ex4, p):
                    for pc in range(2):
                        c = 2 * p + pc
                        for s in range(2):
                            if p == j and pc == 1 and s == 0:
                                continue  # block above the diagonal
                            for h in range(2):
                                nc.tensor.matmul(
                                    av4[:, s, h, 0:65],
                                    ex4[:, pc, h, 128 * s : 128 * s + 128],
                                    v_tiles[c][:, 65 * h : 65 * h + 65],
                                    start=(c == 0 and s == 0 and h == 0),
                                    stop=(c == 2 * j + 1 - (1 - s)),
                                    skip_group_check=True,
                                )

                for p in range(j + 1):
                    # scores for key chunks 2p, 2p+1 (both heads)
                    st = stp.tile([128, 2 * 2 * TQ], f32)
                    st4 = st.rearrange("p (c h n) -> p c h n", c=2, h=2)
                    for pc in range(2):
                        c = 2 * p + pc
                        n0 = 128 if (p == j and pc == 1) else 0
                        jj, cc = divmod(c, 2)
                        for h in range(2):
                            # h0 starts (bank-wide zero); h1 lands on
                            # pending-zero bytes of the same bank
                            nc.tensor.matmul(
                                st4[:, pc, h, n0:TQ],
                                kt_tiles[jj][h][:, 128 * cc : 128 * cc + 128],
                                qt[h][:, n0:TQ],
                                start=(h == 0), stop=True,
                                skip_group_check=True,
                            )
                    ex = exp_p.tile([128, 2 * 2 * TQ], bf16)
                    ex4 = ex.rearrange("p (c h n) -> p c h n", c=2, h=2)
                    if p == j:
                        nc.scalar.activation(
                            ex4[:, 0, :, :], st4[:, 0, :, :], EXP, scale=0.125
                        )
                        nc.scalar.activation(
                            ex4[:, 1, :, 128:TQ], st4[:, 1, :, 128:TQ],
                            EXP, scale=0.125,
                        )
                        # mask the two diagonal blocks per head
                        for h in range(2):
                            nc.vector.tensor_mul(
                                ex4[:, 0, h, 0:128], ex4[:, 0, h, 0:128], mask[:]
                            )
                            nc.vector.tensor_mul(
                                ex4[:, 1, h, 128:TQ], ex4[:, 1, h, 128:TQ], mask[:]
                            )
                    else:
                        nc.scalar.activation(ex[:], st[:], EXP, scale=0.125)

                    prev_ex.append((ex4, p))
                    if len(prev_ex) > 2:
                        emit_av(*prev_ex.pop(0))
                    # spread interleaved units out; force-drain near tile end
                    if pending and (p % 2 == 0 or len(pending) >= (j - p)):
                        pending.pop(0)()

                while prev_ex:
                    emit_av(*prev_ex.pop(0))
                while pending:
                    pending.pop(0)()

                on = onp.tile([128, 2 * 2 * 64], bf16)
                pending = finish_units(j, on, av)

            while pending:
                pending.pop(0)()

    nc.compile()
    return nc


def _prep_inputs(x, w_qkv, b_qkv, w_out, b_out):
    x = np.asarray(x, dtype=np.float32).reshape(T, E)
    w_qkv = np.asarray(w_qkv, dtype=np.float32)
    b_qkv = np.asarray(b_qkv, dtype=np.float32)
    w_out = np.asarray(w_out, dtype=np.float32)
    b_out = np.asarray(b_out, dtype=np.float32)

    xT = np.ascontiguousarray(x.T).astype(ml_dtypes.bfloat16)
    mask = np.triu(np.ones((128, 128), dtype=np.float32)).astype(ml_dtypes.bfloat16)
    ident = np.eye(128, dtype=np.float32).astype(ml_dtypes.bfloat16)

    in_maps = []
    for cidx in range(NCORES):
        lo, hi = 128 * cidx, 128 * (cidx + 1)
        wq = w_qkv[:, lo:hi]
        wk = w_qkv[:, E + lo : E + hi]
        wv = w_qkv[:, 2 * E + lo : 2 * E + hi]
        wqkv_c = np.ascontiguousarray(
            np.concatenate([wq, wk, wv], axis=1)
        ).astype(ml_dtypes.bfloat16)
        in_maps.append(
            {
                "xT": xT,
                "wqkv": wqkv_c,
                "bq2": np.ascontiguousarray(b_qkv[lo:hi].reshape(2, 64).T),
                "wo": np.ascontiguousarray(w_out[lo:hi, :]).astype(
                    ml_dtypes.bfloat16
                ),
                "mask": mask,
                "ident": ident,
            }
        )
    # host-side constant: b_out plus the exact b_v contribution
    b_v = b_qkv[2 * E : 3 * E]
    const_row = b_out + b_v @ w_out
    return in_maps, const_row


def kernel(x, w_qkv, b_qkv, w_out, b_out):
    global LAST_RESULTS
    from concourse.bass_utils import run_bass_kernel_spmd

    if "nc" not in _CACHE:
        _CACHE["nc"] = _build()
    nc = _CACHE["nc"]

    in_maps, const_row = _prep_inputs(x, w_qkv, b_qkv, w_out, b_out)
    res = run_bass_kernel_spmd(nc, in_maps, core_ids=list(range(NCORES)))
    LAST_RESULTS = res

    out = np.zeros((T, E), dtype=np.float32)
    for r in res.results:
        out += r["y"].astype(np.float32)
    out += const_row[None, :].astype(np.float32)
    return out.reshape(1, T, E)
